# revision 1
# baseline (speedup 1.0000x reference)
"""Causal single-head attention (B=4, T=2048, D=1024) on 8 NeuronCores.

Sharding: 2 cores per batch element. Within a batch, core r (r in {0,1})
handles the strided query rows q_glob = 2*j + r (j = 0..1023). The strided
split makes the causal block structure identical on every core (SPMD-safe)
and balances causal work exactly.

Per-core device program (all matmuls bf16 with fp32 PSUM accumulation):
  1. K^T = Wk x^T   [d_out partition-major, 2048 keys]
  2. V   = x Wv^T   [2048 keys partition-major, d_out free]
  3. Q^T = Wq xq^T  [d_out partition-major, 1024 local queries]
  4. per 128-query block: S = Q K^T (fp32 PSUM), additive causal mask on the
     diagonal 512-chunk, exp on ACT (scale 1/32) with accum_out row-sums,
     PE-transpose of P, out2 += P^T V accumulated in PSUM, final per-row
     divide by the softmax denominator.

Host side transposes/bf16-casts inputs and de-interleaves outputs.
"""
import orjson
import numpy as np
import ml_dtypes

import concourse.bass as bass
import concourse.mybir as mybir
import concourse.tile as tile
from concourse import bass_utils
from concourse.masks import make_identity

B, T, D = 4, 2048, 1024
NCORES = 8
P = 128
JQ = T // 2            # local queries per core (1024)
N_QB = JQ // P         # 8 query blocks of 128
N_IT = D // P          # 8 contraction tiles
N_KT = T // P          # 16 key tiles of 128
KC = 512               # key chunk for S matmuls
N_KC = T // KC         # 4
F32 = mybir.dt.float32
BF16 = mybir.dt.bfloat16
SCALE = 1.0 / 32.0     # 1/sqrt(D)
MASK_NEG = -1.0e9

def _split_waits(blocks):
    """The walrus build in this container accepts at most ONE sync-wait per
    instruction; Tile freely emits several. Split extras onto same-engine
    NoOps inserted immediately before the instruction (engine-serial order
    preserves semantics)."""
    n_split = 0
    for blk in blocks:
        insts = blk.get("instructions", [])
        out = []
        for ins in insts:
            si = ins.get("sync_info")
            waits = (si or {}).get("on_wait") or []
            if len(waits) > 1:
                for i, w in enumerate(waits[:-1]):
                    nop = {
                        "engine": ins["engine"],
                        "ins": [],
                        "name": f"{ins['name']}-w{i}",
                        "opcode": "NoOp",
                        "outs": [],
                        "sync_info": {"on_wait": [w], "on_update": []},
                    }
                    if "debug" in ins:
                        nop["debug"] = ins["debug"]
                    out.append(nop)
                    n_split += 1
                si["on_wait"] = [waits[-1]]
            out.append(ins)
        blk["instructions"] = out
        _split_waits(blk.get("blocks", []) or [])
    return n_split


class _Bass(bass.Bass):
    def to_json_bytes(self):
        d = orjson.loads(super().to_json_bytes())
        for f in d["functions"]:
            _split_waits(f["blocks"])
        return orjson.dumps(d)


def n_kc_of(qb: int) -> int:
    # chunks of 512 keys needed by query block qb (covers q_glob < 256*(qb+1))
    return qb // 2 + 1


# Pair-split K/V projection + AllGather exchange trades ~55us of PE work for
# a ~100us (measured) pair-gather of 4 MiB: roughly perf-neutral on HW and
# worse on the cost model, so it stays off. The code path is kept for tuning.
SPLIT_KV = False


def build_nc(split_kv: bool = SPLIT_KV) -> bass.Bass:
    nc = _Bass("TRN2", debug=False, num_devices=NCORES)

    TH = T // 2 if split_kv else T
    # split_kv: core r of a pair owns keys [r*1024, r*1024+1024); K/V are
    # computed for the half then pair-AllGathered. Otherwise each core
    # computes K/V for all keys from the full x^T.
    xhT = nc.dram_tensor("xhT", [D, TH], BF16, kind="ExternalInput")
    xqT = nc.dram_tensor("xqT", [D, JQ], BF16, kind="ExternalInput")
    wqT = nc.dram_tensor("wqT", [D, D], BF16, kind="ExternalInput")
    wkT = nc.dram_tensor("wkT", [D, D], BF16, kind="ExternalInput")
    wvT = nc.dram_tensor("wvT", [D, D], BF16, kind="ExternalInput")
    maskadd = nc.dram_tensor("maskadd", [2, P, KC], F32, kind="ExternalInput")
    out = nc.dram_tensor("out", [JQ, D], F32, kind="ExternalOutput")

    with tile.TileContext(nc) as tc:
        with (
            tc.tile_pool(name="big", bufs=1) as big,
            tc.tile_pool(name="wpool", bufs=2) as wpool,
            tc.tile_pool(name="small", bufs=2) as small,
            tc.tile_pool(name="pwork", bufs=4) as pwork,
            tc.tile_pool(name="ptwork", bufs=8) as ptwork,
            tc.tile_pool(name="dram", bufs=1, space="DRAM") as dram,
            tc.tile_pool(name="mm", bufs=2, space="PSUM") as mm,
            tc.tile_pool(name="ptp", bufs=2, space="PSUM") as ptp,
            tc.tile_pool(name="o2p", bufs=2, space="PSUM") as o2p,
        ):
            # ---- constants ----
            ident = big.tile([P, P], BF16, tag="ident")
            make_identity(nc, ident[:])

            def load_w(dram, chunk=D):
                w = wpool.tile([P, N_IT, D], BF16, tag="w")
                w_r = dram.rearrange("(it p) o -> it p o", p=P)
                for oc0 in range(0, D, chunk):
                    for it in range(N_IT):
                        nc.sync.dma_start(
                            w[:, it, oc0:oc0 + chunk], w_r[it][:, oc0:oc0 + chunk]
                        )
                return w

            # wk first and chunked col-major: the opening K-projection group
            # only needs the first ot columns of every it slice
            wk = load_w(wkT)

            # local keys, chunked tc-major so the tc-outer K loop starts early
            xh = big.tile([P, N_IT, TH], BF16, tag="xh")
            xhT_r = xhT.rearrange("(it p) t -> it p t", p=P)
            for tc4 in range(TH // KC):
                for it in range(N_IT):
                    nc.sync.dma_start(
                        xh[:, it, tc4 * KC:(tc4 + 1) * KC],
                        xhT_r[it][:, tc4 * KC:(tc4 + 1) * KC],
                    )

            # ---- resident K^T / V / Q^T for the attention phase ----
            kt_sb = big.tile([P, N_IT, T], BF16, tag="kt")
            v_sb = big.tile([P, N_KT, D], BF16, tag="v")
            qt_sb = big.tile([P, N_IT, JQ], BF16, tag="qt")

            if split_kv:
                # DRAM bounce buffers for the pair AllGather of local K^T / V
                inb = dram.tile([2, TH, D], BF16, tag="inb", name="inb")
                outb = dram.tile([2, 2, TH, D], BF16, tag="outb", name="outb")

            # ---- K^T_local[o, t] = sum_i WkT[i,o] * xhT[i,t] ----
            for tc4 in range(TH // KC):
                for ot in range(N_IT):
                    acc = mm.tile([P, KC], F32, tag="mm512")
                    for it in range(N_IT):
                        nc.tensor.matmul(
                            acc[:],
                            wk[:, it, ot * P:(ot + 1) * P],
                            xh[:, it, tc4 * KC:(tc4 + 1) * KC],
                            start=(it == 0), stop=(it == N_IT - 1),
                        )
                    if split_kv:
                        stg = pwork.tile([P, KC], BF16, tag="kvst", name="stg")
                        nc.vector.tensor_copy(stg[:], acc[:])
                        nc.sync.dma_start(
                            inb[0, ot * P:(ot + 1) * P, tc4 * KC:(tc4 + 1) * KC], stg[:]
                        )
                    else:
                        nc.vector.tensor_copy(
                            kt_sb[:, ot, tc4 * KC:(tc4 + 1) * KC], acc[:]
                        )

            # ---- V_local[t, o] = sum_i xhT[i,t] * WvT[i,o] ----
            wv = load_w(wvT)
            xq = big.tile([P, N_IT, JQ], BF16, tag="xq")
            xqT_r = xqT.rearrange("(it p) t -> it p t", p=P)
            for it in range(N_IT):
                nc.sync.dma_start(xq[:, it, :], xqT_r[it])
            for tt in range(TH // P):
                for oc in range(D // KC):
                    acc = mm.tile([P, KC], F32, tag="mm512")
                    for it in range(N_IT):
                        nc.tensor.matmul(
                            acc[:],
                            xh[:, it, tt * P:(tt + 1) * P],
                            wv[:, it, oc * KC:(oc + 1) * KC],
                            start=(it == 0), stop=(it == N_IT - 1),
                        )
                    if split_kv:
                        stg = pwork.tile([P, KC], BF16, tag="kvst", name="stg")
                        nc.vector.tensor_copy(stg[:], acc[:])
                        nc.sync.dma_start(
                            inb[1, tt * P:(tt + 1) * P, oc * KC:(oc + 1) * KC], stg[:]
                        )
                    else:
                        nc.vector.tensor_copy(
                            v_sb[:, tt, oc * KC:(oc + 1) * KC], acc[:]
                        )

            if split_kv:
                # ---- pair AllGather: both halves of K^T and V ----
                nc.gpsimd.collective_compute(
                    "AllGather",
                    mybir.AluOpType.bypass,
                    replica_groups=[[0, 1], [2, 3], [4, 5], [6, 7]],
                    ins=[inb.opt()],
                    outs=[outb.opt()],
                )
                for h in range(2):
                    for ot in range(N_IT):
                        nc.sync.dma_start(
                            kt_sb[:, ot, h * TH:(h + 1) * TH],
                            outb[h, 0, ot * P:(ot + 1) * P, :],
                        )
                for tt16 in range(N_KT):
                    h, tl = tt16 // (TH // P), tt16 % (TH // P)
                    nc.sync.dma_start(
                        v_sb[:, tt16, :],
                        outb[h, 1, tl * P:(tl + 1) * P, :],
                    )

            # ---- Q^T projection: Q^T[o, j] = sum_i WqT[i,o] * xqT[i,j] ----
            wq = load_w(wqT)
            for ot in range(N_IT):
                for jc in range(JQ // KC):
                    acc = mm.tile([P, KC], F32, tag="mm512")
                    for it in range(N_IT):
                        nc.tensor.matmul(
                            acc[:],
                            wq[:, it, ot * P:(ot + 1) * P],
                            xq[:, it, jc * KC:(jc + 1) * KC],
                            start=(it == 0), stop=(it == N_IT - 1),
                        )
                    nc.vector.tensor_copy(qt_sb[:, ot, jc * KC:(jc + 1) * KC], acc[:])

            # ---- attention, software-pipelined over (qb, kc) ----
            masks = big.tile([P, 2, KC], F32, tag="masks")
            # DRAM [2,128,512] -> partition-major per mask
            nc.sync.dma_start(masks[:], maskadd.rearrange("m p f -> p m f"))

            pairs = [(qb, kc) for qb in range(N_QB) for kc in range(n_kc_of(qb))]

            state = {}  # per live qb: dict(out2, lparts)

            def width_of(qb, kc):
                # even qb's diagonal chunk only reaches 256 keys in
                return 256 if (qb % 2 == 0 and kc == qb // 2) else KC

            def emit_s_phase(qb, kc):
                nkc = n_kc_of(qb)
                w = width_of(qb, kc)
                if kc == 0:
                    state[qb] = {
                        "out2": o2p.tile([P, D], F32, tag="out2", name="out2"),
                        "lparts": small.tile([P, 2 * N_KC], F32, tag="lparts", name="lparts"),
                        "nl": 0,
                    }
                s_psum = mm.tile([P, KC], F32, tag="mm512")
                for ot in range(N_IT):
                    nc.tensor.matmul(
                        s_psum[:, :w],
                        qt_sb[:, ot, qb * P:(qb + 1) * P],
                        kt_sb[:, ot, kc * KC:kc * KC + w],
                        start=(ot == 0), stop=(ot == N_IT - 1),
                    )
                if kc == nkc - 1:  # diagonal chunk: additive causal mask
                    nc.vector.tensor_add(s_psum[:, :w], s_psum[:, :w], masks[:, qb % 2, :w])
                # exp in 256-wide halves: the first PE transpose can start
                # while ACT still works on the second half
                p_sb = pwork.tile([P, KC], BF16, tag="p")
                for h in range(w // 256):
                    nl = state[qb]["nl"]
                    nc.scalar.activation(
                        p_sb[:, h * 256:(h + 1) * 256],
                        s_psum[:, h * 256:(h + 1) * 256],
                        mybir.ActivationFunctionType.Exp,
                        scale=SCALE, accum_out=state[qb]["lparts"][:, nl:nl + 1],
                    )
                    state[qb]["nl"] = nl + 1
                return p_sb

            def emit_pv_phase(qb, kc, p_sb):
                nkc = n_kc_of(qb)
                w = width_of(qb, kc)
                n_st = w // P
                out2 = state[qb]["out2"]
                for st in range(n_st):
                    ptps = ptp.tile([P, P], BF16, tag="pt")
                    nc.tensor.transpose(ptps[:], p_sb[:, st * P:(st + 1) * P], ident[:])
                    ptk = ptwork.tile([P, P], BF16, tag="ptk")
                    nc.vector.tensor_copy(ptk[:], ptps[:])
                    kt_idx = kc * 4 + st
                    for oc in range(D // KC):
                        nc.tensor.matmul(
                            out2[:, oc * KC:(oc + 1) * KC],
                            ptk[:],
                            v_sb[:, kt_idx, oc * KC:(oc + 1) * KC],
                            start=(kc == 0 and st == 0),
                            stop=(kc == nkc - 1 and st == n_st - 1),
                        )
                if kc == nkc - 1:
                    lparts = state[qb]["lparts"]
                    ltot = small.tile([P, 1], F32, tag="ltot")
                    nc.vector.tensor_reduce(
                        ltot[:], lparts[:, 0:state[qb]["nl"]],
                        axis=mybir.AxisListType.X, op=mybir.AluOpType.add,
                    )
                    linv = small.tile([P, 1], F32, tag="linv")
                    nc.vector.reciprocal(linv[:], ltot[:])
                    for oc in range(D // KC):
                        oh = small.tile([P, KC], F32, tag="oh")
                        nc.vector.tensor_scalar_mul(
                            oh[:], out2[:, oc * KC:(oc + 1) * KC], linv[:]
                        )
                        nc.sync.dma_start(
                            out[qb * P:(qb + 1) * P, oc * KC:(oc + 1) * KC], oh[:]
                        )
                    del state[qb]

            prev = None
            for qb, kc in pairs:
                p_sb = emit_s_phase(qb, kc)
                if prev is not None:
                    emit_pv_phase(*prev)
                prev = (qb, kc, p_sb)
            emit_pv_phase(*prev)

    return nc


_NC = {}


def _get_nc(split_kv: bool = SPLIT_KV):
    if split_kv not in _NC:
        _NC[split_kv] = build_nc(split_kv)
    return _NC[split_kv]


def _prep_in_maps(inputs, Wq, Wk, Wv, split_kv: bool = SPLIT_KV):
    inputs = np.asarray(inputs, dtype=np.float32)
    Wq = np.asarray(Wq, dtype=np.float32)
    Wk = np.asarray(Wk, dtype=np.float32)
    Wv = np.asarray(Wv, dtype=np.float32)

    bf = ml_dtypes.bfloat16
    wqT = np.ascontiguousarray(Wq.T).astype(bf)
    wkT = np.ascontiguousarray(Wk.T).astype(bf)
    wvT = np.ascontiguousarray(Wv.T).astype(bf)

    in_maps = []
    for c in range(NCORES):
        b, r = c // 2, c % 2
        xb = inputs[b]                                  # [T, D]
        if split_kv:
            xhT = np.ascontiguousarray(xb[r * (T // 2):(r + 1) * (T // 2), :].T).astype(bf)
        else:
            xhT = np.ascontiguousarray(xb.T).astype(bf)
        xqT = np.ascontiguousarray(xb[r::2, :].T).astype(bf)  # [D, JQ]
        # additive causal mask for the diagonal 512-chunk:
        # keep (0.0) iff f <= 2p + r + 256*m
        p_idx = np.arange(P)[:, None]
        f_idx = np.arange(KC)[None, :]
        masks = np.empty((2, P, KC), dtype=np.float32)
        for m in range(2):
            keep = f_idx <= 2 * p_idx + r + 256 * m
            masks[m] = np.where(keep, 0.0, MASK_NEG)
        in_maps.append({
            "xhT": xhT, "xqT": xqT,
            "wqT": wqT, "wkT": wkT, "wvT": wvT,
            "maskadd": masks,
        })
    return in_maps


def _gather(res):
    result = np.empty((B, T, D), dtype=np.float32)
    for c in range(NCORES):
        b, r = c // 2, c % 2
        result[b, r::2, :] = res.results[c]["out"]
    return result


def kernel(inputs, Wq, Wk, Wv):
    in_maps = _prep_in_maps(inputs, Wq, Wk, Wv)
    nc = _get_nc()
    res = bass_utils.run_bass_kernel_spmd(nc, in_maps, core_ids=list(range(NCORES)))
    return _gather(res)


def run_traced(inputs, Wq, Wk, Wv):
    """Like kernel() but with NTFF tracing; returns BassKernelResults
    (exec_time_ns, trace path). For test.py only."""
    in_maps = _prep_in_maps(inputs, Wq, Wk, Wv)
    nc = _get_nc()
    res = bass_utils.run_bass_kernel_spmd(
        nc, in_maps, core_ids=list(range(NCORES)), trace=True
    )
    res.full_output = _gather(res)
    return res



# revision 15
# speedup vs baseline: 1.4444x; 1.4444x over previous
"""Causal single-head attention (B=4, T=2048, D=1024) on 8 NeuronCores.

Sharding: 2 cores per batch element. Within a batch, core r (r in {0,1})
handles the strided query rows q_glob = 2*j + r (j = 0..1023). The strided
split makes the causal block structure identical on every core (SPMD-safe)
and balances causal work exactly.

Per-core device program (all matmuls bf16 with fp32 PSUM accumulation):
  1. K^T = Wk x^T   [d_out partition-major, 2048 keys]
  2. V   = x Wv^T   [2048 keys partition-major, d_out free]
  3. Q^T = Wq xq^T  [d_out partition-major, 1024 local queries]
  4. per 128-query block: S = Q K^T (fp32 PSUM), additive causal mask on the
     diagonal 512-chunk, exp on ACT (scale 1/32) with accum_out row-sums,
     PE-transpose of P, out2 += P^T V accumulated in PSUM, final per-row
     divide by the softmax denominator.

Host side transposes/bf16-casts inputs and de-interleaves outputs.
"""
import orjson
import numpy as np
import ml_dtypes

import concourse.bass as bass
import concourse.mybir as mybir
import concourse.tile as tile
from concourse import bass_utils
from concourse.masks import make_identity

B, T, D = 4, 2048, 1024
NCORES = 8
P = 128
JQ = T // 2            # local queries per core (1024)
N_QB = JQ // P         # 8 query blocks of 128
N_IT = D // P          # 8 contraction tiles
N_KT = T // P          # 16 key tiles of 128
KC = 512               # key chunk for S matmuls
N_KC = T // KC         # 4
F32 = mybir.dt.float32
BF16 = mybir.dt.bfloat16
SCALE = 1.0 / 32.0     # 1/sqrt(D)
MASK_NEG = -1.0e9

def _split_waits(blocks):
    """The walrus build in this container accepts at most ONE sync-wait per
    instruction; Tile freely emits several. Split extras onto same-engine
    NoOps inserted immediately before the instruction (engine-serial order
    preserves semantics)."""
    n_split = 0
    for blk in blocks:
        insts = blk.get("instructions", [])
        out = []
        for ins in insts:
            si = ins.get("sync_info")
            waits = (si or {}).get("on_wait") or []
            if len(waits) > 1:
                for i, w in enumerate(waits[:-1]):
                    nop = {
                        "engine": ins["engine"],
                        "ins": [],
                        "name": f"{ins['name']}-w{i}",
                        "opcode": "NoOp",
                        "outs": [],
                        "sync_info": {"on_wait": [w], "on_update": []},
                    }
                    if "debug" in ins:
                        nop["debug"] = ins["debug"]
                    out.append(nop)
                    n_split += 1
                si["on_wait"] = [waits[-1]]
            out.append(ins)
        blk["instructions"] = out
        _split_waits(blk.get("blocks", []) or [])
    return n_split


class _Bass(bass.Bass):
    def to_json_bytes(self):
        d = orjson.loads(super().to_json_bytes())
        for f in d["functions"]:
            _split_waits(f["blocks"])
        return orjson.dumps(d)


def n_kc_of(qb: int) -> int:
    # chunks of 512 keys needed by query block qb (covers q_glob < 256*(qb+1))
    return qb // 2 + 1


# Pair-split K/V projection + AllGather exchange trades ~55us of PE work for
# a ~100us (measured) pair-gather of 4 MiB: roughly perf-neutral on HW and
# worse on the cost model, so it stays off. The code path is kept for tuning.
SPLIT_KV = False
# fp8e4m3 + DoubleRow perf mode on the Q/K projections only: 2 contraction
# subtiles per matmul (virtual 256-deep array), ~1.4x PE throughput on those
# phases. Wq/Wk are pre-scaled x32 into fp8's normal range; the 32*32 factor
# cancels inside the softmax scale. V stays bf16 end-to-end: fp8 V errors
# pass straight to the output (measured 3.6e-2 rel), QK-only is 8e-3.
FP8_PROJ = True
F8 = mybir.dt.float8e4
W_SCALE = 32.0


def build_nc(split_kv: bool = SPLIT_KV, fp8_proj: bool = FP8_PROJ) -> bass.Bass:
    nc = _Bass("TRN2", debug=False, num_devices=NCORES)

    QK_DT = F8 if fp8_proj else BF16
    KSTEP = 2 if fp8_proj else 1
    PMODE = mybir.MatmulPerfMode.DoubleRow if fp8_proj else None
    scale_eff = SCALE / (W_SCALE * W_SCALE) if fp8_proj else SCALE

    TH = T // 2 if split_kv else T
    # split_kv: core r of a pair owns keys [r*1024, r*1024+1024); K/V are
    # computed for the half then pair-AllGathered. Otherwise each core
    # computes K/V for all keys from the full x^T.
    xhT = nc.dram_tensor("xhT", [D, TH], QK_DT, kind="ExternalInput")
    xqT = nc.dram_tensor("xqT", [D, JQ], QK_DT, kind="ExternalInput")
    wqT = nc.dram_tensor("wqT", [D, D], QK_DT, kind="ExternalInput")
    wkT = nc.dram_tensor("wkT", [D, D], QK_DT, kind="ExternalInput")
    wvT = nc.dram_tensor("wvT", [D, D], BF16, kind="ExternalInput")
    if fp8_proj:
        # bf16 copy of the local keys' x^T for the V projection
        xhTb = nc.dram_tensor("xhTb", [D, TH], BF16, kind="ExternalInput")
    maskadd = nc.dram_tensor("maskadd", [2, P, KC], F32, kind="ExternalInput")
    out = nc.dram_tensor("out", [JQ, D], F32, kind="ExternalOutput")

    with tile.TileContext(nc) as tc:
        with (
            tc.tile_pool(name="big", bufs=1) as big,
            tc.tile_pool(name="wpool", bufs=1) as wpool,
            tc.tile_pool(name="small", bufs=2) as small,
            tc.tile_pool(name="pwork", bufs=4) as pwork,
            tc.tile_pool(name="ptwork", bufs=8) as ptwork,
            tc.tile_pool(name="dram", bufs=1, space="DRAM") as dram,
            tc.tile_pool(name="mm", bufs=2, space="PSUM") as mm,
            tc.tile_pool(name="ptp", bufs=2, space="PSUM") as ptp,
            tc.tile_pool(name="o2p", bufs=2, space="PSUM") as o2p,
        ):
            # ---- constants ----
            ident = big.tile([P, P], BF16, tag="ident")
            make_identity(nc, ident[:])

            def load_w(dram, dt, tag, chunk=D):
                w = wpool.tile([P, N_IT, D], dt, tag=tag)
                w_r = dram.rearrange("(it p) o -> it p o", p=P)
                for oc0 in range(0, D, chunk):
                    for it in range(N_IT):
                        nc.sync.dma_start(
                            w[:, it, oc0:oc0 + chunk], w_r[it][:, oc0:oc0 + chunk]
                        )
                return w

            # wk first and chunked col-major: the opening K-projection group
            # only needs the first ot columns of every it slice
            wk = load_w(wkT, QK_DT, "wk")

            # local keys, chunked tc-major so the tc-outer K loop starts early
            xh = big.tile([P, N_IT, TH], QK_DT, tag="xh")
            xhT_r = xhT.rearrange("(it p) t -> it p t", p=P)
            for tc4 in range(TH // KC):
                for it in range(N_IT):
                    nc.sync.dma_start(
                        xh[:, it, tc4 * KC:(tc4 + 1) * KC],
                        xhT_r[it][:, tc4 * KC:(tc4 + 1) * KC],
                    )

            # ---- resident K^T / V / Q^T for the attention phase ----
            kt_sb = big.tile([P, N_IT, T], BF16, tag="kt")
            v_sb = big.tile([P, N_KT, D], BF16, tag="v")
            qt_sb = big.tile([P, N_IT, JQ], BF16, tag="qt")

            if split_kv:
                # DRAM bounce buffers for the pair AllGather of local K^T / V
                inb = dram.tile([2, TH, D], BF16, tag="inb", name="inb")
                outb = dram.tile([2, 2, TH, D], BF16, tag="outb", name="outb")

            # ---- K^T_local[o, t] = sum_i WkT[i,o] * xhT[i,t] ----
            for tc4 in range(TH // KC):
                for ot in range(N_IT):
                    acc = mm.tile([P, KC], F32, tag="mm512")
                    for it in range(0, N_IT, KSTEP):
                        nc.tensor.matmul(
                            acc[:],
                            wk[:, it:it + KSTEP, ot * P:(ot + 1) * P],
                            xh[:, it:it + KSTEP, tc4 * KC:(tc4 + 1) * KC],
                            start=(it == 0), stop=(it == N_IT - KSTEP),
                            perf_mode=PMODE,
                        )
                    if split_kv:
                        stg = pwork.tile([P, KC], BF16, tag="kvst", name="stg")
                        nc.vector.tensor_copy(stg[:], acc[:])
                        nc.sync.dma_start(
                            inb[0, ot * P:(ot + 1) * P, tc4 * KC:(tc4 + 1) * KC], stg[:]
                        )
                    else:
                        nc.vector.tensor_copy(
                            kt_sb[:, ot, tc4 * KC:(tc4 + 1) * KC], acc[:]
                        )

            # ---- V_local[t, o] = sum_i xhT[i,t] * WvT[i,o] ----  (bf16)
            wv = load_w(wvT, BF16, "wv")
            if fp8_proj:
                xhb = big.tile([P, N_IT, TH], BF16, tag="xhb")
                xhTb_r = xhTb.rearrange("(it p) t -> it p t", p=P)
                for tc4 in range(TH // KC):
                    for it in range(N_IT):
                        nc.sync.dma_start(
                            xhb[:, it, tc4 * KC:(tc4 + 1) * KC],
                            xhTb_r[it][:, tc4 * KC:(tc4 + 1) * KC],
                        )
            else:
                xhb = xh
            xq = big.tile([P, N_IT, JQ], QK_DT, tag="xq")
            xqT_r = xqT.rearrange("(it p) t -> it p t", p=P)
            for it in range(N_IT):
                nc.sync.dma_start(xq[:, it, :], xqT_r[it])
            for tt in range(TH // P):
                for oc in range(D // KC):
                    acc = mm.tile([P, KC], F32, tag="mm512")
                    for it in range(N_IT):
                        nc.tensor.matmul(
                            acc[:],
                            xhb[:, it, tt * P:(tt + 1) * P],
                            wv[:, it, oc * KC:(oc + 1) * KC],
                            start=(it == 0), stop=(it == N_IT - 1),
                        )
                    if split_kv:
                        stg = pwork.tile([P, KC], BF16, tag="kvst", name="stg")
                        nc.vector.tensor_copy(stg[:], acc[:])
                        nc.sync.dma_start(
                            inb[1, tt * P:(tt + 1) * P, oc * KC:(oc + 1) * KC], stg[:]
                        )
                    else:
                        nc.vector.tensor_copy(
                            v_sb[:, tt, oc * KC:(oc + 1) * KC], acc[:]
                        )

            if split_kv:
                # ---- pair AllGather: both halves of K^T and V ----
                nc.gpsimd.collective_compute(
                    "AllGather",
                    mybir.AluOpType.bypass,
                    replica_groups=[[0, 1], [2, 3], [4, 5], [6, 7]],
                    ins=[inb.opt()],
                    outs=[outb.opt()],
                )
                for h in range(2):
                    for ot in range(N_IT):
                        nc.sync.dma_start(
                            kt_sb[:, ot, h * TH:(h + 1) * TH],
                            outb[h, 0, ot * P:(ot + 1) * P, :],
                        )
                for tt16 in range(N_KT):
                    h, tl = tt16 // (TH // P), tt16 % (TH // P)
                    nc.sync.dma_start(
                        v_sb[:, tt16, :],
                        outb[h, 1, tl * P:(tl + 1) * P, :],
                    )

            # ---- Q^T projection: Q^T[o, j] = sum_i WqT[i,o] * xqT[i,j] ----
            wq = load_w(wqT, QK_DT, "wq")
            for ot in range(N_IT):
                for jc in range(JQ // KC):
                    acc = mm.tile([P, KC], F32, tag="mm512")
                    for it in range(0, N_IT, KSTEP):
                        nc.tensor.matmul(
                            acc[:],
                            wq[:, it:it + KSTEP, ot * P:(ot + 1) * P],
                            xq[:, it:it + KSTEP, jc * KC:(jc + 1) * KC],
                            start=(it == 0), stop=(it == N_IT - KSTEP),
                            perf_mode=PMODE,
                        )
                    nc.vector.tensor_copy(qt_sb[:, ot, jc * KC:(jc + 1) * KC], acc[:])

            # ---- attention, software-pipelined over (qb, kc) ----
            masks = big.tile([P, 2, KC], F32, tag="masks")
            # DRAM [2,128,512] -> partition-major per mask
            nc.sync.dma_start(masks[:], maskadd.rearrange("m p f -> p m f"))

            pairs = [(qb, kc) for qb in range(N_QB) for kc in range(n_kc_of(qb))]

            state = {}  # per live qb: dict(out2, lparts)

            def width_of(qb, kc):
                # even qb's diagonal chunk only reaches 256 keys in
                return 256 if (qb % 2 == 0 and kc == qb // 2) else KC

            def emit_s_phase(qb, kc):
                nkc = n_kc_of(qb)
                w = width_of(qb, kc)
                if kc == 0:
                    state[qb] = {
                        "out2": o2p.tile([P, D], F32, tag="out2", name="out2"),
                        "lparts": small.tile([P, 2 * N_KC], F32, tag="lparts", name="lparts"),
                        "nl": 0,
                    }
                s_psum = mm.tile([P, KC], F32, tag="mm512")
                for ot in range(N_IT):
                    nc.tensor.matmul(
                        s_psum[:, :w],
                        qt_sb[:, ot, qb * P:(qb + 1) * P],
                        kt_sb[:, ot, kc * KC:kc * KC + w],
                        start=(ot == 0), stop=(ot == N_IT - 1),
                    )
                if kc == nkc - 1:  # diagonal chunk: additive causal mask
                    nc.vector.tensor_add(s_psum[:, :w], s_psum[:, :w], masks[:, qb % 2, :w])
                # exp in 256-wide halves: the first PE transpose can start
                # while ACT still works on the second half
                p_sb = pwork.tile([P, KC], BF16, tag="p")
                for h in range(w // 256):
                    nl = state[qb]["nl"]
                    nc.scalar.activation(
                        p_sb[:, h * 256:(h + 1) * 256],
                        s_psum[:, h * 256:(h + 1) * 256],
                        mybir.ActivationFunctionType.Exp,
                        scale=scale_eff, accum_out=state[qb]["lparts"][:, nl:nl + 1],
                    )
                    state[qb]["nl"] = nl + 1
                return p_sb

            def emit_pv_phase(qb, kc, p_sb):
                nkc = n_kc_of(qb)
                w = width_of(qb, kc)
                n_st = w // P
                out2 = state[qb]["out2"]
                for st in range(n_st):
                    ptps = ptp.tile([P, P], BF16, tag="pt")
                    nc.tensor.transpose(ptps[:], p_sb[:, st * P:(st + 1) * P], ident[:])
                    ptk = ptwork.tile([P, P], BF16, tag="ptk")
                    nc.vector.tensor_copy(ptk[:], ptps[:])
                    kt_idx = kc * 4 + st
                    for oc in range(D // KC):
                        nc.tensor.matmul(
                            out2[:, oc * KC:(oc + 1) * KC],
                            ptk[:],
                            v_sb[:, kt_idx, oc * KC:(oc + 1) * KC],
                            start=(kc == 0 and st == 0),
                            stop=(kc == nkc - 1 and st == n_st - 1),
                        )
                if kc == nkc - 1:
                    lparts = state[qb]["lparts"]
                    ltot = small.tile([P, 1], F32, tag="ltot")
                    nc.vector.tensor_reduce(
                        ltot[:], lparts[:, 0:state[qb]["nl"]],
                        axis=mybir.AxisListType.X, op=mybir.AluOpType.add,
                    )
                    linv = small.tile([P, 1], F32, tag="linv")
                    nc.vector.reciprocal(linv[:], ltot[:])
                    for oc in range(D // KC):
                        oh = small.tile([P, KC], F32, tag="oh")
                        nc.vector.tensor_scalar_mul(
                            oh[:], out2[:, oc * KC:(oc + 1) * KC], linv[:]
                        )
                        nc.sync.dma_start(
                            out[qb * P:(qb + 1) * P, oc * KC:(oc + 1) * KC], oh[:]
                        )
                    del state[qb]

            prev = None
            for qb, kc in pairs:
                p_sb = emit_s_phase(qb, kc)
                if prev is not None:
                    emit_pv_phase(*prev)
                prev = (qb, kc, p_sb)
            emit_pv_phase(*prev)

    return nc


_NC = {}


def _get_nc(split_kv: bool = SPLIT_KV, fp8_proj: bool = FP8_PROJ):
    key = (split_kv, fp8_proj)
    if key not in _NC:
        _NC[key] = build_nc(split_kv, fp8_proj)
    return _NC[key]


def _prep_in_maps(inputs, Wq, Wk, Wv, split_kv: bool = SPLIT_KV,
                  fp8_proj: bool = FP8_PROJ):
    inputs = np.asarray(inputs, dtype=np.float32)
    Wq = np.asarray(Wq, dtype=np.float32)
    Wk = np.asarray(Wk, dtype=np.float32)
    Wv = np.asarray(Wv, dtype=np.float32)

    bf = ml_dtypes.bfloat16
    qk = ml_dtypes.float8_e4m3 if fp8_proj else bf
    ws = np.float32(W_SCALE) if fp8_proj else np.float32(1.0)
    wqT = np.ascontiguousarray(Wq.T * ws).astype(qk)
    wkT = np.ascontiguousarray(Wk.T * ws).astype(qk)
    wvT = np.ascontiguousarray(Wv.T).astype(bf)

    in_maps = []
    for c in range(NCORES):
        b, r = c // 2, c % 2
        xb = inputs[b]                                  # [T, D]
        if split_kv:
            xhTf = np.ascontiguousarray(xb[r * (T // 2):(r + 1) * (T // 2), :].T)
        else:
            xhTf = np.ascontiguousarray(xb.T)
        xhT = xhTf.astype(qk)
        xqT = np.ascontiguousarray(xb[r::2, :].T).astype(qk)  # [D, JQ]
        # additive causal mask for the diagonal 512-chunk:
        # keep (0.0) iff f <= 2p + r + 256*m
        p_idx = np.arange(P)[:, None]
        f_idx = np.arange(KC)[None, :]
        masks = np.empty((2, P, KC), dtype=np.float32)
        for m in range(2):
            keep = f_idx <= 2 * p_idx + r + 256 * m
            masks[m] = np.where(keep, 0.0, MASK_NEG)
        im = {
            "xhT": xhT, "xqT": xqT,
            "wqT": wqT, "wkT": wkT, "wvT": wvT,
            "maskadd": masks,
        }
        if fp8_proj:
            im["xhTb"] = xhTf.astype(bf)
        in_maps.append(im)
    return in_maps


def _gather(res):
    result = np.empty((B, T, D), dtype=np.float32)
    for c in range(NCORES):
        b, r = c // 2, c % 2
        result[b, r::2, :] = res.results[c]["out"]
    return result


def kernel(inputs, Wq, Wk, Wv):
    in_maps = _prep_in_maps(inputs, Wq, Wk, Wv)
    nc = _get_nc()
    res = bass_utils.run_bass_kernel_spmd(nc, in_maps, core_ids=list(range(NCORES)))
    return _gather(res)


def run_traced(inputs, Wq, Wk, Wv):
    """Like kernel() but with NTFF tracing; returns BassKernelResults
    (exec_time_ns, trace path). For test.py only."""
    in_maps = _prep_in_maps(inputs, Wq, Wk, Wv)
    nc = _get_nc()
    res = bass_utils.run_bass_kernel_spmd(
        nc, in_maps, core_ids=list(range(NCORES)), trace=True
    )
    res.full_output = _gather(res)
    return res

